# revision 7
# baseline (speedup 1.0000x reference)
"""CenterLoss forward on 8 Trainium2 NeuronCores.

Reference computation (see problem):
    N = 16*256 = 4096 rows, D = 512, C = 10000 classes
    dist[n] = ||x[n] - centers[labels[n]]||^2
    loss = sum_n clamp(dist[n], 1e-12, 1e12) + N*(C-1)*1e-12
(the constant term comes from the reference clamping the masked-out zero
entries of the full N x C distance matrix to 1e-12 before summing).

Sharding: data-parallel over N. Each of the 8 cores gets 512 rows of x and
labels; centers live (replicated) in each core's DRAM but only the 512
needed rows are read via indirect (gather) DMAs — 20 MB of centers never
moves. x and centers stream as bf16.

Raw bass (no TileContext): the kernel is 12 instructions with a linear
dependency chain, and Tile's exit sequence (drain + all-engine barrier +
sem clear + second barrier) costs ~7 us of measured tail. Hand-rolled
semaphores end the program ~300 ns after the output DMA lands instead.

Per-core layout: shard row r lives at partition r//4, chunk c = r%4
(so every DMA is one instruction with large contiguous descriptors):
 - x arrives as ONE [128, 4*512] bf16 DMA (4 KB contiguous per partition,
   128 descriptors) on the ACT HWDGE ring;
 - labels arrive as ONE [128, 4] int32 DMA (16 B per partition) on the SP
   ring — column c is exactly the [128,1] offset AP chunk c's gather needs
   (the only offset-AP shape the DGE gathers correctly);
 - 4 indirect gathers (gpsimd SWDGE, default ucode lib — dma_gather lives
   in an overlay lib whose mid-kernel load costs ~12 us) land
   centers[label[4p+c]] into g[:, c, :], pipelining with compute;
 - DVE per chunk: subtract (bf16 2x rate) + fused square-and-row-reduce
   (scalar_tensor_tensor, bf16 product tile, f32 accum_out);
 - out [128, 4] f32 (2 KB) on the SP ring; host clamps and reduces in f64
   (out.reshape(-1)[r] is shard row r by construction);
 - gpsimd waits for the out DMA then clears the kernel semaphores (the
   repeat-execution contract Tile's exit normally provides).
"""

import numpy as np

N_CORES = 8
ROWS_TOTAL = 4096
ROWS_PER_CORE = ROWS_TOTAL // N_CORES  # 512
P = 128                                # SBUF partitions
RPP = ROWS_PER_CORE // P               # rows per partition = chunks = 4
D = 512
C = 10000
CLAMP_MIN = 1e-12
CLAMP_MAX = 1e12

_NC_CACHE = {}


def _build_nc():
    from contextlib import ExitStack

    import concourse.bacc as bacc
    import concourse.bass as bass
    from concourse import mybir

    nc = bacc.Bacc("TRN2", target_bir_lowering=False)

    f32 = mybir.dt.float32
    bf16 = mybir.dt.bfloat16
    x_d = nc.dram_tensor("x", [P, RPP * D], bf16, kind="ExternalInput")
    lab_d = nc.dram_tensor("labels", [P, RPP], mybir.dt.int32,
                           kind="ExternalInput")
    cen_d = nc.dram_tensor("centers", [C, D], bf16, kind="ExternalInput")
    out_d = nc.dram_tensor("out", [P, RPP], f32, kind="ExternalOutput")

    with ExitStack() as st:
        lab_t = st.enter_context(nc.sbuf_tensor("lab", [P, RPP],
                                                mybir.dt.int32))
        x_t = st.enter_context(nc.sbuf_tensor("xt", [P, RPP, D], bf16))
        g_t = st.enter_context(nc.sbuf_tensor("gt", [P, RPP, D], bf16))
        rs_t = st.enter_context(nc.sbuf_tensor("rs", [P, RPP], f32))
        d_ts = [st.enter_context(nc.sbuf_tensor(f"d{i}", [P, D], bf16))
                for i in range(2)]
        sq_ts = [st.enter_context(nc.sbuf_tensor(f"sq{i}", [P, D], bf16))
                 for i in range(2)]

        s_lab = nc.alloc_semaphore("s_lab")
        s_x = nc.alloc_semaphore("s_x")
        s_g = [nc.alloc_semaphore(f"s_g{c}") for c in range(RPP)]
        s_v = nc.alloc_semaphore("s_v")
        s_out = nc.alloc_semaphore("s_out")

        # labels first (the gathers' descriptor gen waits on them).
        nc.sync.dma_start(out=lab_t[:, :], in_=lab_d[:, :]).then_inc(s_lab, 16)
        # x as one DMA: 128 x 4 KB contiguous descriptors.
        nc.scalar.dma_start(out=x_t[:, :, :], in_=x_d[:, :]).then_inc(s_x, 16)

        nc.gpsimd.wait_ge(s_lab, 16)
        for c in range(RPP):
            nc.gpsimd.indirect_dma_start(
                out=g_t[:, c, :],
                out_offset=None,
                in_=cen_d[:, :],
                in_offset=bass.IndirectOffsetOnAxis(
                    ap=lab_t[:, c:c + 1], axis=0),
            ).then_inc(s_g[c], 16)

        nc.vector.wait_ge(s_x, 16)
        for c in range(RPP):
            nc.vector.wait_ge(s_g[c], 16)
            d_t, sq_t = d_ts[c % 2], sq_ts[c % 2]
            nc.vector.tensor_sub(d_t[:, :], x_t[:, c, :], g_t[:, c, :])
            # sq = (d + 0) * d, accum_out = per-row sum (f32); fused on DVE
            # (tensor_tensor_reduce hits an unsupported ISA opcode on this
            # runtime). bf16 product tile keeps the 2x DVE rate.
            nc.vector.scalar_tensor_tensor(
                out=sq_t[:, :],
                in0=d_t[:, :],
                scalar=0.0,
                in1=d_t[:, :],
                op0=mybir.AluOpType.add,
                op1=mybir.AluOpType.mult,
                accum_out=rs_t[:, c:c + 1],
            ).then_inc(s_v, 1)

        nc.sync.wait_ge(s_v, RPP)
        nc.sync.dma_start(out=out_d[:, :], in_=rs_t[:, :]).then_inc(s_out, 16)

        # Hold the NEFF open until the output write has landed. No explicit
        # sem clear: the compiler's epilogue zeroes every semaphore (S[3..255],
        # one EVENT_SEMAPHORE per sem split across engines) after the final
        # barrier, which also provides the repeat-execution contract.
        nc.gpsimd.wait_ge(s_out, 16)

    nc.finalize()
    return nc


def _get_nc():
    if "nc" not in _NC_CACHE:
        _NC_CACHE["nc"] = _build_nc()
    return _NC_CACHE["nc"]


def _make_in_maps(x, labels, centers):
    import ml_dtypes
    bf16 = ml_dtypes.bfloat16
    xf = np.ascontiguousarray(np.asarray(x).reshape(ROWS_TOTAL, D)
                              .astype(bf16))
    lab = np.ascontiguousarray(
        np.asarray(labels).reshape(ROWS_TOTAL).astype(np.int32))
    cen = np.ascontiguousarray(np.asarray(centers).astype(bf16))

    in_maps = []
    for k in range(N_CORES):
        sl = slice(k * ROWS_PER_CORE, (k + 1) * ROWS_PER_CORE)
        # row r of the shard -> partition r//4, chunk r%4: plain reshape.
        xs = xf[sl].reshape(P, RPP * D)
        ls = lab[sl].reshape(P, RPP)
        in_maps.append({"x": xs, "labels": ls, "centers": cen})
    return in_maps


def _collect(results):
    """Device outputs -> full loss (host clamp + reduce)."""
    # out[p, c] = squared distance of shard row 4p + c -> reshape(-1)
    # restores shard row order; cores are concatenated in row order.
    per_row = np.concatenate(
        [r["out"].reshape(-1) for r in results]).astype(np.float64)
    total = np.clip(per_row, CLAMP_MIN, CLAMP_MAX).sum()
    total += ROWS_TOTAL * (C - 1) * CLAMP_MIN
    return np.asarray(total, dtype=np.float32)


def kernel(x, labels, centers):
    import time
    from concourse.bass_utils import run_bass_kernel_spmd

    nc = _get_nc()
    in_maps = _make_in_maps(x, labels, centers)
    last_err = None
    for attempt in range(3):
        if attempt:
            time.sleep(30)  # transient device errors recover in <1 min
        try:
            res = run_bass_kernel_spmd(nc, in_maps,
                                       core_ids=list(range(N_CORES)))
            return _collect(res.results)
        except Exception as e:  # noqa: BLE001 - retry any runtime failure
            last_err = e
    raise last_err


# revision 10
# speedup vs baseline: 1.0422x; 1.0422x over previous
"""CenterLoss forward on 8 Trainium2 NeuronCores.

Reference computation (see problem):
    N = 16*256 = 4096 rows, D = 512, C = 10000 classes
    dist[n] = ||x[n] - centers[labels[n]]||^2
    loss = sum_n clamp(dist[n], 1e-12, 1e12) + N*(C-1)*1e-12
(the constant term comes from the reference clamping the masked-out zero
entries of the full N x C distance matrix to 1e-12 before summing).

Sharding: data-parallel over N. Each of the 8 cores gets 512 rows of x and
labels; centers live (replicated) in each core's DRAM but only the 512
needed rows are read via indirect (gather) DMAs — 20 MB of centers never
moves. x and centers stream as bf16.

Raw bass (no TileContext): the kernel is 12 instructions with a linear
dependency chain, and Tile's exit sequence (drain + all-engine barrier +
sem clear + second barrier) costs ~7 us of measured tail. Hand-rolled
semaphores end the program ~300 ns after the output DMA lands instead.

Per-core layout: shard row r lives at partition r//4, chunk c = r%4
(so every DMA is one instruction with large contiguous descriptors):
 - x arrives as ONE [128, 4*512] bf16 DMA (4 KB contiguous per partition,
   128 descriptors) on the ACT HWDGE ring;
 - labels arrive as ONE [128, 4] int32 DMA (16 B per partition) on the SP
   ring — column c is exactly the [128,1] offset AP chunk c's gather needs
   (the only offset-AP shape the DGE gathers correctly);
 - 4 indirect gathers (gpsimd SWDGE, default ucode lib — dma_gather lives
   in an overlay lib whose mid-kernel load costs ~12 us) land
   centers[label[4p+c]] into g[:, c, :], pipelining with compute;
 - DVE per chunk: subtract (bf16 2x rate) + fused square-and-row-reduce
   (scalar_tensor_tensor, bf16 product tile, f32 accum_out);
 - out [128, 4] f32 (2 KB) on the SP ring; host clamps and reduces in f64
   (out.reshape(-1)[r] is shard row r by construction);
 - gpsimd waits for the out DMA then clears the kernel semaphores (the
   repeat-execution contract Tile's exit normally provides).
"""

import numpy as np

N_CORES = 8
ROWS_TOTAL = 4096
ROWS_PER_CORE = ROWS_TOTAL // N_CORES  # 512
P = 128                                # SBUF partitions
RPP = ROWS_PER_CORE // P               # rows per partition = chunks = 4
D = 512
C = 10000
CLAMP_MIN = 1e-12
CLAMP_MAX = 1e12

_NC_CACHE = {}


def _build_nc():
    from contextlib import ExitStack

    import concourse.bacc as bacc
    import concourse.bass as bass
    from concourse import mybir

    nc = bacc.Bacc("TRN2", target_bir_lowering=False)

    f32 = mybir.dt.float32
    bf16 = mybir.dt.bfloat16
    x_d = nc.dram_tensor("x", [P, RPP * D], bf16, kind="ExternalInput")
    lab_d = nc.dram_tensor("labels", [P, RPP], mybir.dt.int32,
                           kind="ExternalInput")
    cen_d = nc.dram_tensor("centers", [C, D], bf16, kind="ExternalInput")
    out_d = nc.dram_tensor("out", [P, RPP], f32, kind="ExternalOutput")

    with ExitStack() as st:
        lab_t = st.enter_context(nc.sbuf_tensor("lab", [P, RPP],
                                                mybir.dt.int32))
        x_t = st.enter_context(nc.sbuf_tensor("xt", [P, RPP, D], bf16))
        g_t = st.enter_context(nc.sbuf_tensor("gt", [P, RPP, D], bf16))
        rs_t = st.enter_context(nc.sbuf_tensor("rs", [P, RPP], f32))
        d_ts = [st.enter_context(nc.sbuf_tensor(f"d{i}", [P, D], bf16))
                for i in range(2)]
        sq_ts = [st.enter_context(nc.sbuf_tensor(f"sq{i}", [P, D], bf16))
                 for i in range(2)]

        s_lab = nc.alloc_semaphore("s_lab")
        s_x = nc.alloc_semaphore("s_x")
        s_g = [nc.alloc_semaphore(f"s_g{c}") for c in range(RPP)]
        s_v = nc.alloc_semaphore("s_v")
        s_out = nc.alloc_semaphore("s_out")

        # labels first (the gathers' descriptor gen waits on them).
        nc.sync.dma_start(out=lab_t[:, :], in_=lab_d[:, :]).then_inc(s_lab, 16)
        # x as one DMA: 128 x 4 KB contiguous descriptors.
        nc.scalar.dma_start(out=x_t[:, :, :], in_=x_d[:, :]).then_inc(s_x, 16)

        nc.gpsimd.wait_ge(s_lab, 16)
        for c in range(RPP):
            nc.gpsimd.indirect_dma_start(
                out=g_t[:, c, :],
                out_offset=None,
                in_=cen_d[:, :],
                in_offset=bass.IndirectOffsetOnAxis(
                    ap=lab_t[:, c:c + 1], axis=0),
            ).then_inc(s_g[c], 16)

        nc.vector.wait_ge(s_x, 16)
        for c in range(RPP):
            nc.vector.wait_ge(s_g[c], 16)
            d_t, sq_t = d_ts[c % 2], sq_ts[c % 2]
            nc.vector.tensor_sub(d_t[:, :], x_t[:, c, :], g_t[:, c, :])
            # sq = (d + 0) * d, accum_out = per-row sum (f32); fused on DVE
            # (tensor_tensor_reduce hits an unsupported ISA opcode on this
            # runtime). bf16 product tile keeps the 2x DVE rate.
            nc.vector.scalar_tensor_tensor(
                out=sq_t[:, :],
                in0=d_t[:, :],
                scalar=0.0,
                in1=d_t[:, :],
                op0=mybir.AluOpType.add,
                op1=mybir.AluOpType.mult,
                accum_out=rs_t[:, c:c + 1],
            ).then_inc(s_v, 1)

        nc.sync.wait_ge(s_v, RPP)
        nc.sync.dma_start(out=out_d[:, :], in_=rs_t[:, :]).then_inc(s_out, 16)

        # No explicit end wait or sem clear: the compiler's epilogue (final
        # drains + all-engine barrier + zeroing of every semaphore, ~6.5 us)
        # both flushes the 2 KB out DMA and provides the repeat-execution
        # contract, so the body can end at the out-DMA issue.

    nc.finalize()
    return nc


def _get_nc():
    if "nc" not in _NC_CACHE:
        _NC_CACHE["nc"] = _build_nc()
    return _NC_CACHE["nc"]


def _make_in_maps(x, labels, centers):
    import ml_dtypes
    bf16 = ml_dtypes.bfloat16
    xf = np.ascontiguousarray(np.asarray(x).reshape(ROWS_TOTAL, D)
                              .astype(bf16))
    lab = np.ascontiguousarray(
        np.asarray(labels).reshape(ROWS_TOTAL).astype(np.int32))
    cen = np.ascontiguousarray(np.asarray(centers).astype(bf16))

    in_maps = []
    for k in range(N_CORES):
        sl = slice(k * ROWS_PER_CORE, (k + 1) * ROWS_PER_CORE)
        # row r of the shard -> partition r//4, chunk r%4: plain reshape.
        xs = xf[sl].reshape(P, RPP * D)
        ls = lab[sl].reshape(P, RPP)
        in_maps.append({"x": xs, "labels": ls, "centers": cen})
    return in_maps


def _collect(results):
    """Device outputs -> full loss (host clamp + reduce)."""
    # out[p, c] = squared distance of shard row 4p + c -> reshape(-1)
    # restores shard row order; cores are concatenated in row order.
    per_row = np.concatenate(
        [r["out"].reshape(-1) for r in results]).astype(np.float64)
    total = np.clip(per_row, CLAMP_MIN, CLAMP_MAX).sum()
    total += ROWS_TOTAL * (C - 1) * CLAMP_MIN
    return np.asarray(total, dtype=np.float32)


def kernel(x, labels, centers):
    import time
    from concourse.bass_utils import run_bass_kernel_spmd

    nc = _get_nc()
    in_maps = _make_in_maps(x, labels, centers)
    last_err = None
    for attempt in range(3):
        if attempt:
            time.sleep(30)  # transient device errors recover in <1 min
        try:
            res = run_bass_kernel_spmd(nc, in_maps,
                                       core_ids=list(range(N_CORES)))
            return _collect(res.results)
        except Exception as e:  # noqa: BLE001 - retry any runtime failure
            last_err = e
    raise last_err


# revision 11
# speedup vs baseline: 1.0425x; 1.0003x over previous
"""CenterLoss forward on 8 Trainium2 NeuronCores.

Reference computation (see problem):
    N = 16*256 = 4096 rows, D = 512, C = 10000 classes
    dist[n] = ||x[n] - centers[labels[n]]||^2
    loss = sum_n clamp(dist[n], 1e-12, 1e12) + N*(C-1)*1e-12
(the constant term comes from the reference clamping the masked-out zero
entries of the full N x C distance matrix to 1e-12 before summing).

Sharding: data-parallel over N. Each of the 8 cores gets 512 rows of x and
labels; centers live (replicated) in each core's DRAM but only the 512
needed rows are read via indirect (gather) DMAs — 20 MB of centers never
moves. x and centers stream as bf16.

Raw bass (no TileContext): the kernel is 12 instructions with a linear
dependency chain, and Tile's exit sequence (drain + all-engine barrier +
sem clear + second barrier) costs ~7 us of measured tail. Hand-rolled
semaphores end the program ~300 ns after the output DMA lands instead.

Per-core layout: shard row r lives at partition r//4, chunk c = r%4
(so every DMA is one instruction with large contiguous descriptors):
 - x arrives as ONE [128, 4*512] bf16 DMA (4 KB contiguous per partition,
   128 descriptors) on the ACT HWDGE ring;
 - labels arrive as ONE [128, 4] int32 DMA (16 B per partition) on the SP
   ring — column c is exactly the [128,1] offset AP chunk c's gather needs
   (the only offset-AP shape the DGE gathers correctly);
 - 4 indirect gathers (gpsimd SWDGE, default ucode lib — dma_gather lives
   in an overlay lib whose mid-kernel load costs ~12 us) land
   centers[label[4p+c]] into g[:, c, :], pipelining with compute;
 - DVE per chunk: subtract (bf16 2x rate) + fused square-and-row-reduce
   (scalar_tensor_tensor, bf16 product tile, f32 accum_out);
 - out [128, 4] f32 (2 KB) on the SP ring; host clamps and reduces in f64
   (out.reshape(-1)[r] is shard row r by construction);
 - gpsimd waits for the out DMA then clears the kernel semaphores (the
   repeat-execution contract Tile's exit normally provides).
"""

import numpy as np

N_CORES = 8
ROWS_TOTAL = 4096
ROWS_PER_CORE = ROWS_TOTAL // N_CORES  # 512
P = 128                                # SBUF partitions
RPP = ROWS_PER_CORE // P               # rows per partition = chunks = 4
D = 512
C = 10000
CLAMP_MIN = 1e-12
CLAMP_MAX = 1e12

_NC_CACHE = {}


def _build_nc():
    from contextlib import ExitStack

    import concourse.bacc as bacc
    import concourse.bass as bass
    from concourse import mybir

    nc = bacc.Bacc("TRN2", target_bir_lowering=False)

    f32 = mybir.dt.float32
    bf16 = mybir.dt.bfloat16
    f8 = mybir.dt.float8e3
    x_d = nc.dram_tensor("x", [P, RPP * D], f8, kind="ExternalInput")
    lab_d = nc.dram_tensor("labels", [P, RPP], mybir.dt.int32,
                           kind="ExternalInput")
    cen_d = nc.dram_tensor("centers", [C, D], f8, kind="ExternalInput")
    out_d = nc.dram_tensor("out", [P, RPP], f32, kind="ExternalOutput")

    with ExitStack() as st:
        lab_t = st.enter_context(nc.sbuf_tensor("lab", [P, RPP],
                                                mybir.dt.int32))
        x_t = st.enter_context(nc.sbuf_tensor("xt", [P, RPP, D], f8))
        g_t = st.enter_context(nc.sbuf_tensor("gt", [P, RPP, D], f8))
        rs_t = st.enter_context(nc.sbuf_tensor("rs", [P, RPP], f32))
        d_ts = [st.enter_context(nc.sbuf_tensor(f"d{i}", [P, D], bf16))
                for i in range(2)]
        sq_ts = [st.enter_context(nc.sbuf_tensor(f"sq{i}", [P, D], bf16))
                 for i in range(2)]

        s_lab = nc.alloc_semaphore("s_lab")
        s_x = nc.alloc_semaphore("s_x")
        s_g = [nc.alloc_semaphore(f"s_g{c}") for c in range(RPP)]
        s_v = nc.alloc_semaphore("s_v")
        s_out = nc.alloc_semaphore("s_out")

        # labels first (the gathers' descriptor gen waits on them).
        nc.sync.dma_start(out=lab_t[:, :], in_=lab_d[:, :]).then_inc(s_lab, 16)
        # x as one DMA: 128 x 4 KB contiguous descriptors.
        nc.scalar.dma_start(out=x_t[:, :, :], in_=x_d[:, :]).then_inc(s_x, 16)

        nc.gpsimd.wait_ge(s_lab, 16)
        for c in range(RPP):
            nc.gpsimd.indirect_dma_start(
                out=g_t[:, c, :],
                out_offset=None,
                in_=cen_d[:, :],
                in_offset=bass.IndirectOffsetOnAxis(
                    ap=lab_t[:, c:c + 1], axis=0),
            ).then_inc(s_g[c], 16)

        nc.vector.wait_ge(s_x, 16)
        for c in range(RPP):
            nc.vector.wait_ge(s_g[c], 16)
            d_t, sq_t = d_ts[c % 2], sq_ts[c % 2]
            nc.vector.tensor_sub(d_t[:, :], x_t[:, c, :], g_t[:, c, :])
            # sq = (d + 0) * d, accum_out = per-row sum (f32); fused on DVE
            # (tensor_tensor_reduce hits an unsupported ISA opcode on this
            # runtime). bf16 product tile keeps the 2x DVE rate.
            nc.vector.scalar_tensor_tensor(
                out=sq_t[:, :],
                in0=d_t[:, :],
                scalar=0.0,
                in1=d_t[:, :],
                op0=mybir.AluOpType.add,
                op1=mybir.AluOpType.mult,
                accum_out=rs_t[:, c:c + 1],
            ).then_inc(s_v, 1)

        nc.sync.wait_ge(s_v, RPP)
        nc.sync.dma_start(out=out_d[:, :], in_=rs_t[:, :]).then_inc(s_out, 16)

        # No explicit end wait or sem clear: the compiler's epilogue (final
        # drains + all-engine barrier + zeroing of every semaphore, ~6.5 us)
        # both flushes the 2 KB out DMA and provides the repeat-execution
        # contract, so the body can end at the out-DMA issue.

    nc.finalize()
    return nc


def _get_nc():
    if "nc" not in _NC_CACHE:
        _NC_CACHE["nc"] = _build_nc()
    return _NC_CACHE["nc"]


def _make_in_maps(x, labels, centers):
    import ml_dtypes
    f8 = ml_dtypes.float8_e3m4
    xf = np.ascontiguousarray(np.asarray(x).reshape(ROWS_TOTAL, D)
                              .astype(f8))
    lab = np.ascontiguousarray(
        np.asarray(labels).reshape(ROWS_TOTAL).astype(np.int32))
    cen = np.ascontiguousarray(np.asarray(centers).astype(f8))

    in_maps = []
    for k in range(N_CORES):
        sl = slice(k * ROWS_PER_CORE, (k + 1) * ROWS_PER_CORE)
        # row r of the shard -> partition r//4, chunk r%4: plain reshape.
        xs = xf[sl].reshape(P, RPP * D)
        ls = lab[sl].reshape(P, RPP)
        in_maps.append({"x": xs, "labels": ls, "centers": cen})
    return in_maps


def _collect(results):
    """Device outputs -> full loss (host clamp + reduce)."""
    # out[p, c] = squared distance of shard row 4p + c -> reshape(-1)
    # restores shard row order; cores are concatenated in row order.
    per_row = np.concatenate(
        [r["out"].reshape(-1) for r in results]).astype(np.float64)
    total = np.clip(per_row, CLAMP_MIN, CLAMP_MAX).sum()
    total += ROWS_TOTAL * (C - 1) * CLAMP_MIN
    return np.asarray(total, dtype=np.float32)


def kernel(x, labels, centers):
    import time
    from concourse.bass_utils import run_bass_kernel_spmd

    nc = _get_nc()
    in_maps = _make_in_maps(x, labels, centers)
    last_err = None
    for attempt in range(3):
        if attempt:
            time.sleep(30)  # transient device errors recover in <1 min
        try:
            res = run_bass_kernel_spmd(nc, in_maps,
                                       core_ids=list(range(N_CORES)))
            return _collect(res.results)
        except Exception as e:  # noqa: BLE001 - retry any runtime failure
            last_err = e
    raise last_err


# revision 13
# speedup vs baseline: 1.0791x; 1.0351x over previous
"""CenterLoss forward on 8 Trainium2 NeuronCores.

Reference computation (see problem):
    N = 16*256 = 4096 rows, D = 512, C = 10000 classes
    dist[n] = ||x[n] - centers[labels[n]]||^2
    loss = sum_n clamp(dist[n], 1e-12, 1e12) + N*(C-1)*1e-12
(the constant term comes from the reference clamping the masked-out zero
entries of the full N x C distance matrix to 1e-12 before summing).

Sharding: data-parallel over N. Each of the 8 cores gets 512 rows of x and
labels; centers live (replicated) in each core's DRAM but only the 512
needed rows are read via indirect (gather) DMAs — 20 MB of centers never
moves. x and centers stream as bf16.

Raw bass (no TileContext): the kernel is 12 instructions with a linear
dependency chain, and Tile's exit sequence (drain + all-engine barrier +
sem clear + second barrier) costs ~7 us of measured tail. Hand-rolled
semaphores end the program ~300 ns after the output DMA lands instead.

Per-core layout: shard row r lives at partition r//4, chunk c = r%4
(so every DMA is one instruction with large contiguous descriptors):
 - x arrives as ONE [128, 4*512] bf16 DMA (4 KB contiguous per partition,
   128 descriptors) on the ACT HWDGE ring;
 - labels arrive as ONE [128, 4] int32 DMA (16 B per partition) on the SP
   ring — column c is exactly the [128,1] offset AP chunk c's gather needs
   (the only offset-AP shape the DGE gathers correctly);
 - 4 indirect gathers (gpsimd SWDGE, default ucode lib — dma_gather lives
   in an overlay lib whose mid-kernel load costs ~12 us) land
   centers[label[4p+c]] into g[:, c, :], pipelining with compute;
 - DVE per chunk: subtract (bf16 2x rate) + fused square-and-row-reduce
   (scalar_tensor_tensor, bf16 product tile, f32 accum_out);
 - out [128, 4] f32 (2 KB) on the SP ring; host clamps and reduces in f64
   (out.reshape(-1)[r] is shard row r by construction);
 - gpsimd waits for the out DMA then clears the kernel semaphores (the
   repeat-execution contract Tile's exit normally provides).
"""

import numpy as np

N_CORES = 8
ROWS_TOTAL = 4096
ROWS_PER_CORE = ROWS_TOTAL // N_CORES  # 512
P = 128                                # SBUF partitions
RPP = ROWS_PER_CORE // P               # rows per partition = chunks = 4
D = 512
C = 10000
CLAMP_MIN = 1e-12
CLAMP_MAX = 1e12

_NC_CACHE = {}


def _build_nc():
    from contextlib import ExitStack

    import concourse.bacc as bacc
    import concourse.bass as bass
    from concourse import mybir

    nc = bacc.Bacc("TRN2", target_bir_lowering=False)

    f32 = mybir.dt.float32
    bf16 = mybir.dt.bfloat16
    x_d = nc.dram_tensor("x", [P, RPP * D], bf16, kind="ExternalInput")
    lab_d = nc.dram_tensor("labels", [P, RPP], mybir.dt.int32,
                           kind="ExternalInput")
    cen_d = nc.dram_tensor("centers", [C, D], bf16, kind="ExternalInput")
    out_d = nc.dram_tensor("out", [P, RPP], f32, kind="ExternalOutput")

    with ExitStack() as st:
        lab_t = st.enter_context(nc.sbuf_tensor("lab", [P, RPP],
                                                mybir.dt.int32))
        x_t = st.enter_context(nc.sbuf_tensor("xt", [P, RPP, D], bf16))
        g_t = st.enter_context(nc.sbuf_tensor("gt", [P, RPP, D], bf16))
        rs_t = st.enter_context(nc.sbuf_tensor("rs", [P, RPP], f32))
        d_ts = [st.enter_context(nc.sbuf_tensor(f"d{i}", [P, D], bf16))
                for i in range(2)]
        sq_ts = [st.enter_context(nc.sbuf_tensor(f"sq{i}", [P, D], bf16))
                 for i in range(2)]

        s_lab = nc.alloc_semaphore("s_lab")
        s_x = nc.alloc_semaphore("s_x")
        s_g = [nc.alloc_semaphore(f"s_g{c}") for c in range(RPP)]
        s_v = nc.alloc_semaphore("s_v")
        s_out = nc.alloc_semaphore("s_out")

        # labels first (the gathers' descriptor gen waits on them).
        nc.sync.dma_start(out=lab_t[:, :], in_=lab_d[:, :]).then_inc(s_lab, 16)
        # x as one DMA: 128 x 4 KB contiguous descriptors.
        nc.scalar.dma_start(out=x_t[:, :, :], in_=x_d[:, :]).then_inc(s_x, 16)

        nc.gpsimd.wait_ge(s_lab, 16)
        for c in range(RPP):
            nc.gpsimd.indirect_dma_start(
                out=g_t[:, c, :],
                out_offset=None,
                in_=cen_d[:, :],
                in_offset=bass.IndirectOffsetOnAxis(
                    ap=lab_t[:, c:c + 1], axis=0),
            ).then_inc(s_g[c], 16)

        nc.vector.wait_ge(s_x, 16)
        for c in range(RPP):
            nc.vector.wait_ge(s_g[c], 16)
            d_t, sq_t = d_ts[c % 2], sq_ts[c % 2]
            nc.vector.tensor_sub(d_t[:, :], x_t[:, c, :], g_t[:, c, :])
            # sq = (d + 0) * d, accum_out = per-row sum (f32); fused on DVE
            # (tensor_tensor_reduce hits an unsupported ISA opcode on this
            # runtime). bf16 product tile keeps the 2x DVE rate.
            nc.vector.scalar_tensor_tensor(
                out=sq_t[:, :],
                in0=d_t[:, :],
                scalar=0.0,
                in1=d_t[:, :],
                op0=mybir.AluOpType.add,
                op1=mybir.AluOpType.mult,
                accum_out=rs_t[:, c:c + 1],
            ).then_inc(s_v, 1)

        nc.sync.wait_ge(s_v, RPP)
        nc.sync.dma_start(out=out_d[:, :], in_=rs_t[:, :]).then_inc(s_out, 16)

        # No explicit end wait or sem clear: the compiler's epilogue (final
        # drains + all-engine barrier + zeroing of every semaphore, ~6.5 us)
        # both flushes the 2 KB out DMA and provides the repeat-execution
        # contract, so the body can end at the out-DMA issue.

    # Hoist the two input DMA issues above the framework's const-memset
    # barrier: they read nothing the barrier fences (DRAM inputs -> fresh
    # SBUF tiles), and issuing them before the SP/ACT engines park at the
    # barrier overlaps their ~1.3 us issue+DGE+sem latency with the barrier
    # release, so the gathers' label wait clears ~0.7 us earlier.
    blk = nc.m.functions[0].blocks[0]
    insts = blk.instructions
    first_drain = next(i for i, ins in enumerate(insts)
                       if type(ins).__name__ == "InstDrain")
    dmas = [ins for ins in insts
            if type(ins).__name__ == "InstDMACopy"
            and ins.engine in (mybir.EngineType.SP,
                               mybir.EngineType.Activation)][:2]
    assert len(dmas) == 2
    for ins in dmas:
        insts.remove(ins)
    for j, ins in enumerate(dmas):
        insts.insert(first_drain + j, ins)

    nc.finalize()
    return nc


def _get_nc():
    if "nc" not in _NC_CACHE:
        _NC_CACHE["nc"] = _build_nc()
    return _NC_CACHE["nc"]


def _make_in_maps(x, labels, centers):
    import ml_dtypes
    bf16 = ml_dtypes.bfloat16
    xf = np.ascontiguousarray(np.asarray(x).reshape(ROWS_TOTAL, D)
                              .astype(bf16))
    lab = np.ascontiguousarray(
        np.asarray(labels).reshape(ROWS_TOTAL).astype(np.int32))
    cen = np.ascontiguousarray(np.asarray(centers).astype(bf16))

    in_maps = []
    for k in range(N_CORES):
        sl = slice(k * ROWS_PER_CORE, (k + 1) * ROWS_PER_CORE)
        # row r of the shard -> partition r//4, chunk r%4: plain reshape.
        xs = xf[sl].reshape(P, RPP * D)
        ls = lab[sl].reshape(P, RPP)
        in_maps.append({"x": xs, "labels": ls, "centers": cen})
    return in_maps


def _collect(results):
    """Device outputs -> full loss (host clamp + reduce)."""
    # out[p, c] = squared distance of shard row 4p + c -> reshape(-1)
    # restores shard row order; cores are concatenated in row order.
    per_row = np.concatenate(
        [r["out"].reshape(-1) for r in results]).astype(np.float64)
    total = np.clip(per_row, CLAMP_MIN, CLAMP_MAX).sum()
    total += ROWS_TOTAL * (C - 1) * CLAMP_MIN
    return np.asarray(total, dtype=np.float32)


def kernel(x, labels, centers):
    import time
    from concourse.bass_utils import run_bass_kernel_spmd

    nc = _get_nc()
    in_maps = _make_in_maps(x, labels, centers)
    last_err = None
    for attempt in range(3):
        if attempt:
            time.sleep(30)  # transient device errors recover in <1 min
        try:
            res = run_bass_kernel_spmd(nc, in_maps,
                                       core_ids=list(range(N_CORES)))
            return _collect(res.results)
        except Exception as e:  # noqa: BLE001 - retry any runtime failure
            last_err = e
    raise last_err
